# revision 1
# baseline (speedup 1.0000x reference)
"""Locally-connected 2D conv (unshared weights), VALID, stride 2 — Trainium2 Bass kernel.

Problem (hardcoded):
  x:       (16, 32, 113, 113) f32
  weights: (56, 56, 32, 3, 3, 64) f32   (H_out, W_out, C_in, kh, kw, C_out)
  bias:    (56, 56, 64) f32
  out:     (16, 64, 56, 56) f32
  out[b,o,u,v] = sum_{c,q,r} x[b,c,2u+q,2v+r] * weights[u,v,c,q,r,o] + bias[u,v,o]

Sharding: H_out split across 8 cores (7 output rows each). Each core reads only
its contiguous 1/8 slice of the 231MB weight tensor (the dominant traffic).

Per-core kernel formulation: for each output location (u,v), a matmul
  out(o=64, b=16) = W(K, o).T @ patches(K, b),  K = (c,q) = 96 partitions,
with the kw offset r handled as 3 PSUM-accumulated matmuls (weights tile free
dim carries (v, r, o)), and bias folded in as a 97th contraction row against an
all-ones patch row. One PSUM accumulation group spans a whole 28-v bank chunk
(start on first matmul, stop on last; first write per byte range overwrites).
"""

import numpy as np

B = 16
C_IN = 32
C_OUT = 64
H_OUT = 56
W_OUT = 56
KK = 3
STRIDE = 2
H_IN = 113

N_CORES = 8
U_PER = H_OUT // N_CORES          # 7 output rows per core
ROWS_IN = (U_PER - 1) * STRIDE + KK  # 15 input rows per core
J_ROWS = ROWS_IN - (KK - 1)       # 13 rows stored per q-shifted copy
VCHUNK = 28                       # output cols per PSUM bank chunk
XFREE = B * J_ROWS * H_IN         # x' tile free size (f32 elems)
WFREE = VCHUNK * KK * C_OUT       # weight chunk free size
KPART = C_IN * KK                 # 96 contraction partitions (c,q)

_CACHE = {}


def _build():
    import concourse.mybir as mybir
    from concourse import bacc
    from concourse.tile import TileContext

    f32 = mybir.dt.float32
    nc = bacc.Bacc("TRN2", target_bir_lowering=False, debug=False,
                   num_devices=N_CORES)
    x_in = nc.dram_tensor("x", [B, C_IN, ROWS_IN, H_IN], f32,
                          kind="ExternalInput").ap()
    w_in = nc.dram_tensor("w", [U_PER, W_OUT, C_IN, KK, KK, C_OUT], f32,
                          kind="ExternalInput").ap()
    b_in = nc.dram_tensor("b", [U_PER, W_OUT, C_OUT], f32,
                          kind="ExternalInput").ap()
    y_out = nc.dram_tensor("y", [B, C_OUT, U_PER, W_OUT], f32,
                           kind="ExternalOutput").ap()

    with TileContext(nc) as tc:
        with tc.tile_pool(name="xpool", bufs=1) as xpool, \
             tc.tile_pool(name="wpool", bufs=3) as wpool, \
             tc.tile_pool(name="opool", bufs=1) as opool, \
             tc.tile_pool(name="pspool", bufs=4, space="PSUM") as pspool:

            # x' tile: partition p = q*32 + c holds x[b, c, q+j, w]; row 96 = ones
            xt = xpool.tile([KPART + 1, XFREE], f32)
            nc.vector.memset(xt[KPART:KPART + 1, :], 1.0)
            xt3 = xt.rearrange("p (b hw) -> p b hw", b=B)
            for q in range(KK):
                xsrc = x_in[:, :, q:q + J_ROWS, :].rearrange("b c h w -> c b (h w)")
                nc.sync.dma_start(out=xt3[32 * q:32 * q + 32], in_=xsrc)

            # output staging: partition o, free (b, u, v) -> contiguous dest runs
            out_all = opool.tile([C_OUT, B * U_PER * W_OUT], f32)
            oa3 = out_all.rearrange("p (b uv) -> p b uv", b=B)

            for u in range(U_PER):
                for v0 in range(0, W_OUT, VCHUNK):
                    # weight chunk: partition p=(q,c), free (v, r, o); bias on row 96
                    wt = wpool.tile([KPART + 1, WFREE], f32)
                    wt3 = wt.rearrange("p (v ro) -> p v ro", v=VCHUNK)
                    for q in range(KK):
                        wsrc = w_in[u, v0:v0 + VCHUNK, :, q, :, :].rearrange(
                            "v c r o -> c v (r o)")
                        nc.sync.dma_start(out=wt3[32 * q:32 * q + 32], in_=wsrc)
                    nc.sync.dma_start(out=wt3[KPART:KPART + 1, :, 0:C_OUT],
                                      in_=b_in[u:u + 1, v0:v0 + VCHUNK, :])

                    ps = pspool.tile([C_OUT, VCHUNK * B], f32)
                    for vl in range(VCHUNK):
                        v = v0 + vl
                        for r in range(KK):
                            kk = KPART + 1 if r == 0 else KPART
                            lhsT = wt3[0:kk, vl:vl + 1,
                                       r * C_OUT:(r + 1) * C_OUT]
                            col = (2 * u) * H_IN + STRIDE * v + r
                            rhs = xt3[0:kk, :, col:col + 1]
                            nc.tensor.matmul(
                                ps[:, vl * B:(vl + 1) * B], lhsT, rhs,
                                start=(vl == 0 and r == 0),
                                stop=(vl == VCHUNK - 1 and r == KK - 1),
                            )
                    ps3 = ps.rearrange("p (v b) -> p b v", v=VCHUNK)
                    nc.vector.tensor_copy(
                        oa3[:, :, u * W_OUT + v0:u * W_OUT + v0 + VCHUNK], ps3)

            ydst = y_out.rearrange("b o u v -> o b (u v)")
            nc.sync.dma_start(out=ydst, in_=out_all.rearrange(
                "p (b uv) -> p b uv", b=B))

    nc.compile()
    return nc


def _get_nc():
    if "nc" not in _CACHE:
        _CACHE["nc"] = _build()
    return _CACHE["nc"]


def kernel(x, weights, bias, _trace=False):
    from concourse.bass_utils import run_bass_kernel_spmd

    x = np.ascontiguousarray(x, dtype=np.float32)
    weights = np.ascontiguousarray(weights, dtype=np.float32)
    bias = np.ascontiguousarray(bias, dtype=np.float32)

    nc = _get_nc()
    core_ids = list(range(N_CORES))
    in_maps = []
    for i in core_ids:
        u0 = i * U_PER
        in_maps.append({
            "x": np.ascontiguousarray(
                x[:, :, STRIDE * u0:STRIDE * u0 + ROWS_IN, :]),
            "w": np.ascontiguousarray(weights[u0:u0 + U_PER]),
            "b": np.ascontiguousarray(bias[u0:u0 + U_PER]),
        })
    res = run_bass_kernel_spmd(nc, in_maps, core_ids, trace=_trace)
    out = np.concatenate([res.results[i]["y"] for i in core_ids], axis=2)
    if _trace:
        _CACHE["last_result"] = res
    return out
